# revision 7
# baseline (speedup 1.0000x reference)
"""Trainium2 Bass kernel for BaselineProtonet (retrieval_knn).

logits[q, c] = -||query_q - proto_c||_2
  proto_c = mean of 64 support embeddings of class c
  embeddings_stacked: [64 classes * (64 support + 64 query), 1024] f32

Sharding (8 cores): 2D grid, 4 query-groups x 2 class-groups. Core
(a, b) owns query rows 1024a..1024(a+1) and classes 32b..32b+32.
Per-core wire traffic: 2 MB queries (bf16) + 2 MB support (fp8) + 8 KB
one-hot = 4 MB (vs 5.25 MB for the 1D query-shard baseline), and the
prototype matmul work halves. The per-core logits tile is [32 classes,
1024 queries]; the host stitches the 4x2 grid (layout/encoding only).

Per core:
  protos  : 16 fp8 DoubleRow one-hot matmuls (one-hot stationary per
            chunk-pair, support streams 512 cols) -> p_ps [32, 1024] f32
  W       : ACT evac p_ps/64 -> bf16, 8 PE transposes, ACT scale -2
            -> W bf16 [128 d, 8, 32 c]
  ||p||^2 : ACT square-accumulate on the evacuated protos -> [32,1] f32,
            summed on DVE -> sqrt bias (per-partition = per-class)
  ||q||^2 : bf16 DVE squares per d-chunk tracking the query stream;
            summed over d by all-ones matmuls into the Gram PSUM group
  Gram    : 16 bf16 matmuls lhsT=W chunk (load hidden by 512-col
            stream), rhs=Q^T chunk
  logits  : -sqrt(dist^2) via ACT sqrt(+bias) and DVE negate, two
            pipelined query halves; output [32, 1024] f32 class-major.
DMAs are spread across the three DMA-capable queues (scalar/sync HWDGE,
gpsimd SWDGE); the one-hot rides at the head of the support tensor. PE
is pre-warmed with dummy matmuls (HAM clock gate) and the sqrt/square
ACT tables are preloaded by dummy activations. PE program order places
query-gated matmuls last so the in-order engine queue never blocks the
prototype/Gram chain on late query chunks.
"""

import numpy as np

C = 64          # classes
S = 64          # support per class (== queries per class)
D = 1024        # embedding dim
NCORES = 8
AQ = 4          # query groups
BC = 2          # class groups
CL = C // BC                # 32 classes per core
QL = (C * S) // AQ          # 1024 query rows per core
DCH = D // 128              # 8 d-chunks
SCH = (CL * S) // 128       # 16 support row chunks per core
NJP = SCH // 2              # 8 DoubleRow chunk-pairs
OHW = NJP * 2 * CL          # one-hot words (512 fp8 = 4 cols of f32)

_CACHE = {}


def _emit(nc, tc, sup, qt, out):
    """Emit the per-core tile program.

    sup: [128, OHW + SCH*D] fp8 DRAM (one-hot header + support of this
         core's 32 classes, swizzled: row p of chunk j = support row
         j*128+p)
    qt:  [128, DCH*QL] bf16 DRAM    (queries, swizzled feature-major)
    out: [CL, QL] f32 DRAM          (negated distances, class-major)
    """
    from concourse import masks, mybir

    f32 = mybir.dt.float32
    bf16 = mybir.dt.bfloat16
    fp8 = mybir.dt.float8e4
    AF = mybir.ActivationFunctionType

    with (
        tc.tile_pool(name="sb", bufs=1) as sb,
        tc.tile_pool(name="ps", bufs=1, space="PSUM") as ps,
    ):
        # warm the PE clock first-thing (HAM gate needs ~3us of busy
        # before the real matmuls; deps are a single DVE memset)
        wm_in = sb.tile([128, 512], bf16)
        nc.vector.memset(wm_in[:], 0.0)
        wm_ps = ps.tile([128, 512], f32)
        for _ in range(7):
            nc.tensor.matmul(
                wm_ps[:], wm_in[:, 0:128], wm_in[:], start=True, stop=True
            )

        # ---------------- input DMAs --------------------------------
        # sc8 = one-hot header + support; pieces of 4 chunks (512 KB)
        sc8 = sb.tile([128, OHW + SCH * D], fp8)
        q16 = sb.tile([128, DCH, QL], bf16)

        def sup_piece(eng, lo, hi):
            eng.dma_start(sc8[:, lo:hi], sup[:, lo:hi])

        def q_piece(eng, h):
            eng.dma_start(
                q16[:, 2 * h : 2 * (h + 1)],
                qt[:, 2 * h * QL : 2 * (h + 1) * QL].rearrange(
                    "p (k q) -> p k q", k=2
                ),
            )

        E = OHW
        sup_piece(nc.sync, 0, E + 8 * D)            # one-hot + chunks 0-7
        sup_piece(nc.sync, E + 8 * D, E + 16 * D)   # chunks 8-15
        for h in range(4):                          # query chunk pairs
            q_piece(nc.gpsimd, h)

        oh = sc8[:, 0:OHW].rearrange("p (jp o c) -> p jp o c", jp=NJP, o=2)
        s8v = sc8[:, OHW:].rearrange("p (jp o d) -> p jp o d", jp=NJP, o=2)

        # ---------------- constants (DVE: it cannot issue DMAs) ------
        ident = sb.tile([128, 128], bf16)
        masks.make_identity(nc, ident[:])
        ones16 = sb.tile([128, CL], bf16)
        nc.vector.memset(ones16[:], 1.0)

        # preload the sqrt+square ACT tables off the critical path
        warm_sq = sb.tile([1, 2], f32)
        nc.vector.memset(warm_sq[:], 1.0)
        nc.scalar.activation(warm_sq[:, 0:1], warm_sq[:, 0:1], AF.Sqrt)
        nc.scalar.activation(warm_sq[:, 1:2], warm_sq[:, 1:2], AF.Square)

        # ---------------- prototypes [class, d] ----------------------
        # chunk-pair jp = 256 support rows = classes 4jp..4jp+4; one-hot
        # is stationary, support streams; fp8 DoubleRow.
        p_ps = ps.tile([CL, D], f32)
        for jp in range(NJP):
            for h in range(2):
                nc.tensor.matmul(
                    p_ps[:, 512 * h : 512 * (h + 1)],
                    oh[:, jp],
                    s8v[:, jp, :, 512 * h : 512 * (h + 1)],
                    start=(jp == 0),
                    stop=(jp == NJP - 1),
                    perf_mode=mybir.MatmulPerfMode.DoubleRow,
                )

        # ---------------- ||q||^2 squares (DVE, bf16 2x) -------------
        qsq = sb.tile([128, DCH, QL], bf16)
        for k in range(DCH):
            nc.vector.tensor_mul(qsq[:, k], q16[:, k], q16[:, k])

        # early ||q||^2 matmuls open the two s_ps PSUM bank groups and
        # track the query stream while the W chain completes (k-major so
        # each chunk's pair of matmuls fires as soon as its square lands)
        s_ps = ps.tile([CL, QL], f32)
        for k in range(4):
            for h in range(2):
                nc.tensor.matmul(
                    s_ps[:, 512 * h : 512 * (h + 1)],
                    ones16[:],
                    qsq[:, k, 512 * h : 512 * (h + 1)],
                    start=(k == 0),
                    stop=False,
                )

        # ---------------- W chain (ACT + PE transposes) --------------
        # evacuate p/64 in two halves (separate tiles so the transposes
        # can start on half A while half B evacuates)
        psbA = sb.tile([CL, 512], bf16)
        psbB = sb.tile([CL, 512], bf16)
        nc.scalar.mul(psbA[:], p_ps[:, 0:512], 1.0 / S)
        nc.scalar.mul(psbB[:], p_ps[:, 512:1024], 1.0 / S)

        ptp = ps.tile([128, DCH, CL], bf16)
        for k in range(DCH):
            half = psbA if k < 4 else psbB
            nc.tensor.transpose(
                ptp[:, k],
                half[:, 128 * (k % 4) : 128 * (k % 4 + 1)],
                ident[0:CL, 0:CL],
            )
        W = sb.tile([128, DCH, CL], bf16)
        nc.scalar.mul(W[:], ptp[:], -2.0)

        # ||p||^2 via ACT square-accumulate on the evacuated protos
        pn_dump = sb.tile([CL, D], bf16)
        pnA = sb.tile([CL, 1], f32)
        pnB = sb.tile([CL, 1], f32)
        pn_col = sb.tile([CL, 1], f32)
        nc.scalar.activation(pn_dump[:, 0:512], psbA[:], AF.Square, accum_out=pnA[:])
        nc.scalar.activation(pn_dump[:, 512:1024], psbB[:], AF.Square, accum_out=pnB[:])
        nc.vector.tensor_add(pn_col[:], pnA[:], pnB[:])

        # ---------------- Gram + late ||q||^2 ------------------------
        # Gram k tracks query chunk k (W load hidden by 512-col stream);
        # the k>=4 ||q||^2 matmuls close the groups after the last
        # squares land.
        for k in range(DCH):
            for h in range(2):
                nc.tensor.matmul(
                    s_ps[:, 512 * h : 512 * (h + 1)],
                    W[:, k],
                    q16[:, k, 512 * h : 512 * (h + 1)],
                    start=False,
                    stop=False,
                )
        for k in range(4, DCH):
            for h in range(2):
                nc.tensor.matmul(
                    s_ps[:, 512 * h : 512 * (h + 1)],
                    ones16[:],
                    qsq[:, k, 512 * h : 512 * (h + 1)],
                    start=False,
                    stop=(k == DCH - 1),
                )

        # ------- sqrt(+||p||^2), negate, store (2 halves pipelined) --
        lt = sb.tile([CL, QL], f32)
        for h in range(2):
            s = slice(512 * h, 512 * (h + 1))
            nc.scalar.activation(lt[:, s], s_ps[:, s], AF.Sqrt, bias=pn_col[:, 0:1])
            nc.vector.tensor_scalar_mul(lt[:, s], lt[:, s], -1.0)
            nc.scalar.dma_start(out[:, s], lt[:, s])


def _build():
    if "nc" in _CACHE:
        return _CACHE["nc"]
    from concourse import bacc, mybir, tile

    f32 = mybir.dt.float32
    bf16 = mybir.dt.bfloat16
    fp8 = mybir.dt.float8e4
    nc = bacc.Bacc(
        "TRN2",
        target_bir_lowering=False,
        debug=False,
        enable_asserts=False,
        num_devices=NCORES,
    )
    sup = nc.dram_tensor(
        "sup", [128, OHW + SCH * D], fp8, kind="ExternalInput"
    ).ap()
    qt = nc.dram_tensor("qt", [128, DCH * QL], bf16, kind="ExternalInput").ap()
    out = nc.dram_tensor("out", [CL, QL], f32, kind="ExternalOutput").ap()
    with tile.TileContext(nc) as tc:
        _emit(nc, tc, sup, qt, out)
    nc.compile()
    _CACHE["nc"] = nc
    return nc


def _onehot():
    import ml_dtypes

    # oh[p, jp, o, c] = 1 iff class c owns support row (2jp+o)*128+p,
    # i.e. c == 4jp + 2o + p//64
    p = np.arange(128)[:, None, None, None]
    jp = np.arange(NJP)[None, :, None, None]
    o = np.arange(2)[None, None, :, None]
    c = np.arange(CL)[None, None, None, :]
    oh = (c == 4 * jp + 2 * o + p // 64).astype(ml_dtypes.float8_e4m3)
    return oh.reshape(128, OHW)


def _shard(embeddings):
    import ml_dtypes

    emb = np.asarray(embeddings, dtype=np.float32).reshape(C, 2 * S, D)
    oh = _onehot()
    # support per class-group b: one-hot header + swizzled [128, SCH, D]
    sups = []
    for b in range(BC):
        sb = emb[CL * b : CL * (b + 1), :S, :].reshape(SCH, 128, D)
        sb = sb.transpose(1, 0, 2).astype(ml_dtypes.float8_e4m3)
        sups.append(
            np.ascontiguousarray(
                np.concatenate([oh, sb.reshape(128, SCH * D)], axis=1)
            )
        )
    # queries per query-group a: Q^T swizzled [128, DCH, QL] bf16
    query_set = emb[:, S:, :].reshape(C * S, D)
    qts = []
    for a in range(AQ):
        q = query_set[QL * a : QL * (a + 1)]
        qt_a = q.T.reshape(DCH, 128, QL).transpose(1, 0, 2)
        qts.append(
            np.ascontiguousarray(
                qt_a.astype(ml_dtypes.bfloat16).reshape(128, DCH * QL)
            )
        )
    in_maps = []
    for i in range(NCORES):
        a, b = divmod(i, BC)
        in_maps.append({"sup": sups[b], "qt": qts[a]})
    return in_maps


def _gather(outs):
    """Stitch per-core [CL, QL] blocks into full [C*S, C] logits."""
    logits = np.empty((C * S, C), dtype=np.float32)
    for i in range(NCORES):
        a, b = divmod(i, BC)
        logits[QL * a : QL * (a + 1), CL * b : CL * (b + 1)] = (
            np.asarray(outs[i], dtype=np.float32).T
        )
    return logits


def kernel(embeddings_stacked, n_classes, n_support, **_unused):
    assert int(n_classes) == C and int(n_support) == S
    emb = np.asarray(embeddings_stacked)
    assert emb.shape == (C * 2 * S, D), emb.shape

    from concourse import bass_utils

    nc = _build()
    in_maps = _shard(emb)
    try:
        res = bass_utils.run_bass_kernel_spmd(
            nc, in_maps, core_ids=list(range(NCORES))
        )
    except Exception:
        # transient device/runtime hiccups have been observed; retry once
        res = bass_utils.run_bass_kernel_spmd(
            nc, in_maps, core_ids=list(range(NCORES))
        )
    return _gather([res.results[i]["out"] for i in range(NCORES)])


if __name__ == "__main__":
    rng = np.random.default_rng(0)
    emb = rng.standard_normal((C * 2 * S, D), dtype=np.float32)
    got = kernel(emb, C, S)
    print("kernel output", got.shape, got.dtype)


# revision 8
# speedup vs baseline: 1.0681x; 1.0681x over previous
"""Trainium2 Bass kernel for BaselineProtonet (retrieval_knn).

logits[q, c] = -||query_q - proto_c||_2
  proto_c = mean of 64 support embeddings of class c
  embeddings_stacked: [64 classes * (64 support + 64 query), 1024] f32

Sharding (8 cores): 2D grid, 4 query-groups x 2 class-groups. Core
(a, b) owns query rows 1024a..1024(a+1) and classes 32b..32b+32.
Per-core wire traffic: 2 MB queries (bf16) + 2 MB support (fp8) + 8 KB
one-hot = 4 MB (vs 5.25 MB for the 1D query-shard baseline), and the
prototype matmul work halves. The per-core logits tile is [32 classes,
1024 queries]; the host stitches the 4x2 grid (layout/encoding only).

Per core:
  protos  : 16 fp8 DoubleRow one-hot matmuls (one-hot stationary per
            chunk-pair, support streams 512 cols) -> p_ps [32, 1024] f32
  W       : per-d-half pipeline: ACT evac p/64 -> bf16, 4 PE
            transposes, ACT scale -2 -> W bf16 [128 d, 8, 32 c]; the
            k<4 half of the Gram can start while the k>=4 half of the
            chain is still in flight
  ||p||^2 : ACT square-accumulate on the evacuated protos -> [32,1] f32,
            summed on DVE -> sqrt bias (per-partition = per-class)
  ||q||^2 : bf16 DVE squares per d-chunk tracking the query stream;
            summed over d by all-ones matmuls into the Gram PSUM group
  Gram    : 16 bf16 matmuls lhsT=W chunk (load hidden by 512-col
            stream), rhs=Q^T chunk; Gram k0 opens the PSUM groups
  logits  : -sqrt(dist^2) via ACT sqrt(+bias) and DVE negate, two
            pipelined query halves; output [32, 1024] f32 class-major.

Support rides the sync HWDGE queue (starts immediately), queries ride
the gpsimd SWDGE queue, outputs ride the scalar queue (whose head-of-
stream ACT table loads then cost nothing). The PE instruction stream is
ordered explicitly with tile_wait_until ranks so the in-order engine
never blocks early work (prototypes) on late query data. PE is
pre-warmed with dummy matmuls (HAM clock gate).
"""

import numpy as np

C = 64          # classes
S = 64          # support per class (== queries per class)
D = 1024        # embedding dim
NCORES = 8
AQ = 4          # query groups
BC = 2          # class groups
CL = C // BC                # 32 classes per core
QL = (C * S) // AQ          # 1024 query rows per core
DCH = D // 128              # 8 d-chunks
SCH = (CL * S) // 128       # 16 support row chunks per core
NJP = SCH // 2              # 8 DoubleRow chunk-pairs
OHW = NJP * 2 * CL          # one-hot bytes per partition

_CACHE = {}


def _emit(nc, tc, sup, qt, out):
    """Emit the per-core tile program.

    sup: [128, OHW + SCH*D] fp8 DRAM (one-hot header + support of this
         core's 32 classes, swizzled: row p of chunk j = support row
         j*128+p)
    qt:  [128, DCH*QL] bf16 DRAM    (queries, swizzled feature-major)
    out: [CL, QL] f32 DRAM          (negated distances, class-major)
    """
    from concourse import masks, mybir

    f32 = mybir.dt.float32
    bf16 = mybir.dt.bfloat16
    fp8 = mybir.dt.float8e4
    AF = mybir.ActivationFunctionType

    with (
        tc.tile_pool(name="sb", bufs=1) as sb,
        tc.tile_pool(name="ps", bufs=1, space="PSUM") as ps,
    ):
        # warm the PE clock first-thing (HAM gate needs ~3us of busy
        # before the real matmuls; deps are a single DVE memset)
        wm_in = sb.tile([128, 512], bf16)
        nc.vector.memset(wm_in[:], 0.0)
        wm_ps = ps.tile([128, 512], f32)
        for _ in range(7):
            nc.tensor.matmul(
                wm_ps[:], wm_in[:, 0:128], wm_in[:], start=True, stop=True
            )

        # ---------------- input DMAs --------------------------------
        sc8 = sb.tile([128, OHW + SCH * D], fp8)
        q16 = sb.tile([128, DCH, QL], bf16)

        E = OHW
        nc.sync.dma_start(sc8[:, 0 : E + 8 * D], sup[:, 0 : E + 8 * D])
        nc.sync.dma_start(sc8[:, E + 8 * D :], sup[:, E + 8 * D :])
        for h in range(4):
            nc.gpsimd.dma_start(
                q16[:, 2 * h : 2 * (h + 1)],
                qt[:, 2 * h * QL : 2 * (h + 1) * QL].rearrange(
                    "p (k q) -> p k q", k=2
                ),
            )

        oh = sc8[:, 0:OHW].rearrange("p (jp o c) -> p jp o c", jp=NJP, o=2)
        s8v = sc8[:, OHW:].rearrange("p (jp o d) -> p jp o d", jp=NJP, o=2)

        # ---------------- constants (DVE: it cannot issue DMAs) ------
        ident = sb.tile([128, 128], bf16)
        masks.make_identity(nc, ident[:])
        ones16 = sb.tile([128, CL], bf16)
        nc.vector.memset(ones16[:], 1.0)

        # preload the sqrt+square ACT tables off the critical path
        warm_sq = sb.tile([1, 2], f32)
        nc.vector.memset(warm_sq[:], 1.0)
        nc.scalar.activation(warm_sq[:, 0:1], warm_sq[:, 0:1], AF.Sqrt)
        nc.scalar.activation(warm_sq[:, 1:2], warm_sq[:, 1:2], AF.Square)

        # ---------------- prototypes [class, d] ----------------------
        # chunk-pair jp = 256 support rows = classes 4jp..4jp+4; one-hot
        # is stationary, support streams; fp8 DoubleRow.
        p_ps = ps.tile([CL, D], f32)
        with tc.tile_wait_until(1):
            for jp in range(NJP):
                for h in range(2):
                    nc.tensor.matmul(
                        p_ps[:, 512 * h : 512 * (h + 1)],
                        oh[:, jp],
                        s8v[:, jp, :, 512 * h : 512 * (h + 1)],
                        start=(jp == 0),
                        stop=(jp == NJP - 1),
                        perf_mode=mybir.MatmulPerfMode.DoubleRow,
                    )

        # ---------------- ||q||^2 squares (DVE, bf16 2x) -------------
        qsq = sb.tile([128, DCH, QL], bf16)
        with tc.tile_wait_until(2):
            for k in range(DCH):
                nc.vector.tensor_mul(qsq[:, k], q16[:, k], q16[:, k])

        # ---------------- W chain, pipelined per d-half --------------
        psbA = sb.tile([CL, 512], bf16)
        psbB = sb.tile([CL, 512], bf16)
        ptp = ps.tile([128, DCH, CL], bf16)
        W = sb.tile([128, DCH, CL], bf16)
        pn_dump = sb.tile([CL, D], bf16)
        pnA = sb.tile([CL, 1], f32)
        pnB = sb.tile([CL, 1], f32)
        pn_col = sb.tile([CL, 1], f32)

        def w_half(hd, psb):
            nc.scalar.mul(psb[:], p_ps[:, 512 * hd : 512 * (hd + 1)], 1.0 / S)
            for k in range(4 * hd, 4 * hd + 4):
                nc.tensor.transpose(
                    ptp[:, k],
                    psb[:, 128 * (k % 4) : 128 * (k % 4 + 1)],
                    ident[0:CL, 0:CL],
                )
            nc.scalar.mul(
                W[:, 4 * hd : 4 * hd + 4], ptp[:, 4 * hd : 4 * hd + 4], -2.0
            )

        with tc.tile_wait_until(3):
            w_half(0, psbA)
        with tc.tile_wait_until(4):
            w_half(1, psbB)
            # ||p||^2 square-accumulates ride after the W scales on ACT
            nc.scalar.activation(
                pn_dump[:, 0:512], psbA[:], AF.Square, accum_out=pnA[:]
            )
            nc.scalar.activation(
                pn_dump[:, 512:1024], psbB[:], AF.Square, accum_out=pnB[:]
            )
            nc.vector.tensor_add(pn_col[:], pnA[:], pnB[:])

        # ------- Gram (opens the PSUM groups) + ||q||^2 matmuls ------
        # interleaved by data readiness: Gram k tracks query piece k//2
        # and the W half k//4; ones k tracks square k.
        s_ps = ps.tile([CL, QL], f32)

        def gram(k, start):
            for h in range(2):
                nc.tensor.matmul(
                    s_ps[:, 512 * h : 512 * (h + 1)],
                    W[:, k],
                    q16[:, k, 512 * h : 512 * (h + 1)],
                    start=start,
                    stop=False,
                )

        def ones_mm(k, stop):
            for h in range(2):
                nc.tensor.matmul(
                    s_ps[:, 512 * h : 512 * (h + 1)],
                    ones16[:],
                    qsq[:, k, 512 * h : 512 * (h + 1)],
                    start=False,
                    stop=stop,
                )

        with tc.tile_wait_until(5):
            gram(0, True)
            gram(1, False)
            ones_mm(0, False)
            ones_mm(1, False)
            gram(2, False)
            gram(3, False)
            ones_mm(2, False)
            ones_mm(3, False)
        with tc.tile_wait_until(6):
            gram(4, False)
            gram(5, False)
            ones_mm(4, False)
            ones_mm(5, False)
            gram(6, False)
            gram(7, False)
            ones_mm(6, False)
            ones_mm(7, True)

        # ------- sqrt(+||p||^2), negate, store (2 halves pipelined) --
        lt = sb.tile([CL, QL], f32)
        with tc.tile_wait_until(7):
            for h in range(2):
                s = slice(512 * h, 512 * (h + 1))
                nc.scalar.activation(
                    lt[:, s], s_ps[:, s], AF.Sqrt, bias=pn_col[:, 0:1]
                )
                nc.vector.tensor_scalar_mul(lt[:, s], lt[:, s], -1.0)
                nc.scalar.dma_start(out[:, s], lt[:, s])


def _build():
    if "nc" in _CACHE:
        return _CACHE["nc"]
    from concourse import bacc, mybir, tile

    f32 = mybir.dt.float32
    bf16 = mybir.dt.bfloat16
    fp8 = mybir.dt.float8e4
    nc = bacc.Bacc(
        "TRN2",
        target_bir_lowering=False,
        debug=False,
        enable_asserts=False,
        num_devices=NCORES,
    )
    sup = nc.dram_tensor(
        "sup", [128, OHW + SCH * D], fp8, kind="ExternalInput"
    ).ap()
    qt = nc.dram_tensor("qt", [128, DCH * QL], bf16, kind="ExternalInput").ap()
    out = nc.dram_tensor("out", [CL, QL], f32, kind="ExternalOutput").ap()
    with tile.TileContext(nc) as tc:
        _emit(nc, tc, sup, qt, out)
    nc.compile()
    _CACHE["nc"] = nc
    return nc


def _onehot():
    import ml_dtypes

    # oh[p, jp, o, c] = 1 iff class c owns support row (2jp+o)*128+p,
    # i.e. c == 4jp + 2o + p//64
    p = np.arange(128)[:, None, None, None]
    jp = np.arange(NJP)[None, :, None, None]
    o = np.arange(2)[None, None, :, None]
    c = np.arange(CL)[None, None, None, :]
    oh = (c == 4 * jp + 2 * o + p // 64).astype(ml_dtypes.float8_e4m3)
    return oh.reshape(128, OHW)


def _shard(embeddings):
    import ml_dtypes

    emb = np.asarray(embeddings, dtype=np.float32).reshape(C, 2 * S, D)
    oh = _onehot()
    # support per class-group b: one-hot header + swizzled [128, SCH, D]
    sups = []
    for b in range(BC):
        sb = emb[CL * b : CL * (b + 1), :S, :].reshape(SCH, 128, D)
        sb = sb.transpose(1, 0, 2).astype(ml_dtypes.float8_e4m3)
        sups.append(
            np.ascontiguousarray(
                np.concatenate([oh, sb.reshape(128, SCH * D)], axis=1)
            )
        )
    # queries per query-group a: Q^T swizzled [128, DCH, QL] bf16
    query_set = emb[:, S:, :].reshape(C * S, D)
    qts = []
    for a in range(AQ):
        q = query_set[QL * a : QL * (a + 1)]
        qt_a = q.T.reshape(DCH, 128, QL).transpose(1, 0, 2)
        qts.append(
            np.ascontiguousarray(
                qt_a.astype(ml_dtypes.bfloat16).reshape(128, DCH * QL)
            )
        )
    in_maps = []
    for i in range(NCORES):
        a, b = divmod(i, BC)
        in_maps.append({"sup": sups[b], "qt": qts[a]})
    return in_maps


def _gather(outs):
    """Stitch per-core [CL, QL] blocks into full [C*S, C] logits."""
    logits = np.empty((C * S, C), dtype=np.float32)
    for i in range(NCORES):
        a, b = divmod(i, BC)
        logits[QL * a : QL * (a + 1), CL * b : CL * (b + 1)] = (
            np.asarray(outs[i], dtype=np.float32).T
        )
    return logits


def kernel(embeddings_stacked, n_classes, n_support, **_unused):
    assert int(n_classes) == C and int(n_support) == S
    emb = np.asarray(embeddings_stacked)
    assert emb.shape == (C * 2 * S, D), emb.shape

    from concourse import bass_utils

    nc = _build()
    in_maps = _shard(emb)
    try:
        res = bass_utils.run_bass_kernel_spmd(
            nc, in_maps, core_ids=list(range(NCORES))
        )
    except Exception:
        # transient device/runtime hiccups have been observed; retry once
        res = bass_utils.run_bass_kernel_spmd(
            nc, in_maps, core_ids=list(range(NCORES))
        )
    return _gather([res.results[i]["out"] for i in range(NCORES)])


if __name__ == "__main__":
    rng = np.random.default_rng(0)
    emb = rng.standard_normal((C * 2 * S, D), dtype=np.float32)
    got = kernel(emb, C, S)
    print("kernel output", got.shape, got.dtype)
